# revision 3
# baseline (speedup 1.0000x reference)
"""ArcFace loss kernel for 8 TRN2 NeuronCores (vocab/tensor-parallel).

reference:
    xn = normalize(x)               # [B, D]
    wn = normalize(weight)          # [C, D]
    logits = 64 * xn @ wn.T         # [B, C]
    loss = mean(CE(logits, label))

Strategy: shard classes C=100000 over 8 cores (12500 each, exact - no
padding). Host prepares normalized, transposed fp8(e4m3) operands scaled
by G=8 (so device cosines are 64*cos and the exp scale is 1); each core
computes its logit shard with TensorE fp8 DoubleRow matmuls (K=256 per
op) into fp32 PSUM and a fused exp+row-sum on ScalarE with a fixed shift
(logsumexp(l) = SHIFT + log(sum(exp(l - SHIFT))), exact since l <= 64).

v2 pipeline notes (from trace analysis of the 77.8us baseline):
  - ScalarE EXP stream is the pacer (~55us busy incl per-op overhead);
    critical path = preamble(5.7us fixed) + time-to-first-EXP + packed
    EXP stream + drain.
  - The 212-class tail chunk is processed as the FIRST group: it needs
    only 106KB of weights, so its 4 small EXPs start ~level with the
    first weight DMAs landing and fill the window while the first full
    1MB group streams in.
  - Warmup matmuls use a K=128 bf16 stationary (not K=1): the TRN2 PE
    p-state governor needs real array utilization to ramp 0.65->2.4GHz,
    and any PE idle gap during the ramp resets it (427ns/MM at MID).
  - DMA descriptor generation costs ~640ns per dma_start on the issuing
    engine; issue is spread over sync/vector/scalar/gpsimd so the first
    group's pieces are all in flight ~2 waves after the preamble.
  - Each core returns raw per-(row, bblock, group) partials [128, 28];
    the host sums group columns, so no on-device reduce/extra sync.
"""

import math
import numpy as np

import concourse.mybir as mybir
import concourse.tile as tile
from concourse import bacc
from concourse.bass_utils import run_bass_kernel_spmd

# Problem constants (hardcoded per harness contract).
B = 512
D = 512
C = 100000
S = 64.0
SHIFT = 20.0  # logsumexp shift; keeps Z ~1e-2 (HW Ln saturates below ~1e-19)
EPS = 1e-12
G = 8.0      # fp8 pre-scale on both operands: device cos' = G^2 * cos
NCORES = 8
CS = C // NCORES        # classes per core = 12500 (exact, no padding)
CHUNK = 512             # matmul moving free dim = one full PSUM bank
GROUP = 4               # psum banks per exp/accumulate group
PB = 128                # partitions
KSUB = D // PB          # 4 contraction subtiles of 128
BBLK = B // PB          # 4 batch blocks
TAIL = CS - 24 * CHUNK  # 212: ragged tail chunk, processed first
N_WARM = 12             # fat PE warm-up matmuls (K=128) to ramp the clock

F32 = mybir.dt.float32
BF16 = mybir.dt.bfloat16
FP8 = mybir.dt.float8e4
NP_FP8 = mybir.dt.np(FP8)
EXP_SCALE = S / (G * G)  # = 1.0

# groups: [tail 212] + six full groups of 4x512. (col0, ncols) per group.
GROUPS = [(24 * CHUNK, TAIL)] + [
    (g * GROUP * CHUNK, GROUP * CHUNK) for g in range(6)
]
NGROUPS = len(GROUPS)  # 7


def build_nc(ncores: int = NCORES):
    """Build the SPMD Bass graph."""
    nc = bacc.Bacc(
        "TRN2",
        target_bir_lowering=False,
        debug=False,
        num_devices=ncores,
    )

    wnt_ext = nc.dram_tensor("wnt", [D, CS], FP8, kind="ExternalInput")
    xnt_ext = nc.dram_tensor("xnt", [D, B], FP8, kind="ExternalInput")
    zp_ext = nc.dram_tensor("zp", [PB, BBLK * NGROUPS], F32, kind="ExternalOutput")

    with tile.TileContext(nc) as tc:
        with (
            tc.tile_pool(name="const", bufs=1) as cpool,
            tc.tile_pool(name="wpool", bufs=7) as wpool,
            tc.tile_pool(name="dpool", bufs=2) as dpool,
        ):
            # ---- SBUF tiles -------------------------------------------
            # exp bias (-SHIFT) as a per-partition vector
            negs = cpool.tile([PB, 1], F32)
            # x^T (normalized, G-scaled) as [128, KSUB, B]: d = ksub*128 + p
            xsb = cpool.tile([PB, KSUB, B], FP8)
            # warmup operands: real K=128 stationary so the PE ramps
            warm_s = cpool.tile([PB, PB], BF16)
            warm_m = cpool.tile([PB, CHUNK], BF16)
            # per (b-block, group) partial row-sums of exp(logit - SHIFT)
            partials = cpool.tile([PB, BBLK * NGROUPS], F32)

            # memsets go on gpsimd (its DMA issues come later); they must
            # not delay the first descriptor-gens on sync/vector/scalar.
            nc.gpsimd.memset(warm_s, 0.0)
            nc.gpsimd.memset(warm_m, 0.0)
            nc.gpsimd.memset(negs, -SHIFT)

            # ---- DMA issue plan ---------------------------------------
            # Priority startup pieces, round-robin over the 3 DMA-capable
            # engines (SP/sync, Activation/scalar, Pool/gpsimd); scalar
            # only participates at startup (it becomes the EXP pacer).
            start_engines = [nc.sync, nc.scalar, nc.gpsimd]
            main_engines = [nc.sync, nc.gpsimd]

            wt_tiles = []
            for col0, ncols in GROUPS:
                wt_tiles.append(
                    wpool.tile(
                        [PB, KSUB, ncols], FP8, name="wt", tag="w",
                        padded_shape=[PB, KSUB, GROUP * CHUNK],
                    )
                )

            def issue_wt(gi, ks, h0, hn, eng):
                col0 = GROUPS[gi][0]
                eng.dma_start(
                    out=wt_tiles[gi][:, ks, h0 : h0 + hn],
                    in_=wnt_ext[
                        ks * PB : (ks + 1) * PB,
                        col0 + h0 : col0 + h0 + hn,
                    ],
                )

            def issue_xsb(ks, b0, bn, eng):
                eng.dma_start(
                    out=xsb[:, ks, b0 : b0 + bn],
                    in_=xnt_ext.rearrange("(ks p) b -> p ks b", p=PB)[
                        :, ks, b0 : b0 + bn
                    ],
                )

            # wave 1-2: tail group weights (106KB) + first xsb batch block
            sq = []
            for ks in range(KSUB):
                sq.append(("wt", 0, ks, 0, TAIL))
            for ks in range(KSUB):
                sq.append(("xsb", ks, 0, PB))
            # waves 3+: group 1 (first full group) in 256-col pieces so the
            # first full-group matmuls unlock ASAP, then remaining xsb.
            for h0 in range(0, GROUP * CHUNK, 256):
                for ks in range(KSUB):
                    sq.append(("wt", 1, ks, h0, 256))
            for ks in range(KSUB):
                sq.append(("xsb", ks, PB, B - PB))

            for i, item in enumerate(sq):
                eng = start_engines[i % len(start_engines)]
                if item[0] == "wt":
                    issue_wt(item[1], item[2], item[3], item[4], eng)
                else:
                    issue_xsb(item[1], item[2], item[3], eng)

            # steady state: groups 2..6 in 1024-col pieces
            dma_i = 0
            for gi in range(2, NGROUPS):
                ncols = GROUPS[gi][1]
                for ks in range(KSUB):
                    for h0 in range(0, ncols, 1024):
                        hn = min(1024, ncols - h0)
                        eng = main_engines[dma_i % len(main_engines)]
                        dma_i += 1
                        issue_wt(gi, ks, h0, hn, eng)

            # ---- compute ----------------------------------------------
            with tc.tile_pool(name="psmain", bufs=2, space="PSUM") as pspool:
                # PE warm-up: full-array (K=128) matmuls so the p-state
                # governor ramps to 2.4GHz before the first real matmul.
                warm_ps = pspool.tile(
                    [PB, GROUP, CHUNK], F32, name="warm_ps", tag="ps",
                )
                for _ in range(N_WARM):
                    nc.tensor.matmul(
                        out=warm_ps[:, 0, :], lhsT=warm_s, rhs=warm_m,
                        start=True, stop=True,
                    )

                for gi, (col0, ncols) in enumerate(GROUPS):
                    nsub = math.ceil(ncols / CHUNK)
                    wt = wt_tiles[gi]
                    for bb in range(BBLK):
                        ps = pspool.tile(
                            [PB, nsub, CHUNK], F32, name="ps", tag="ps",
                            padded_shape=[PB, GROUP, CHUNK],
                        )
                        for k2 in range(KSUB // 2):
                            for sub in range(nsub):
                                cn = min(CHUNK, ncols - sub * CHUNK)
                                nc.tensor.matmul(
                                    out=ps[:, sub, :cn],
                                    lhsT=xsb[
                                        :, 2 * k2 : 2 * k2 + 2,
                                        bb * PB : (bb + 1) * PB,
                                    ],
                                    rhs=wt[
                                        :, 2 * k2 : 2 * k2 + 2,
                                        sub * CHUNK : sub * CHUNK + cn,
                                    ],
                                    start=(k2 == 0),
                                    stop=(k2 == KSUB // 2 - 1),
                                    perf_mode=mybir.MatmulPerfMode.DoubleRow,
                                )
                        dump = dpool.tile(
                            [PB, nsub, CHUNK], BF16, name="dump", tag="dump",
                            padded_shape=[PB, GROUP, CHUNK],
                        )
                        # exp(EXP_SCALE * cos' - SHIFT), accumulated per row.
                        # ragged tail group (nsub=1, 212 cols) reads exactly
                        # its columns; full groups read nsub*512.
                        if ncols % CHUNK == 0:
                            in_ap = ps[:, :, :]
                            out_ap = dump[:, :, :]
                        else:
                            in_ap = ps[:, 0, :ncols]
                            out_ap = dump[:, 0, :ncols]
                        nc.scalar.activation(
                            out=out_ap,
                            in_=in_ap,
                            func=mybir.ActivationFunctionType.Exp,
                            bias=negs,
                            scale=EXP_SCALE,
                            accum_out=partials[
                                :, bb * NGROUPS + gi : bb * NGROUPS + gi + 1
                            ],
                        )

            # raw partials out (host sums the group columns per row)
            nc.sync.dma_start(out=zp_ext[:], in_=partials)

    nc.finalize()
    return nc


def prepare_inputs(x, weight, label, ncores: int = NCORES):
    """Host-side prep: normalize, transpose, G-scale, cast fp8, shard.

    Returns (in_maps, lc2) where lc2[p, j] = SHIFT - S*cos(x_b, w_label_b)
    for b = j*128 + p."""
    x = np.asarray(x, dtype=np.float32)
    weight = np.asarray(weight, dtype=np.float32)
    label = np.asarray(label).astype(np.int64)

    xn = x / np.maximum(
        np.sqrt(np.einsum("bd,bd->b", x, x, dtype=np.float64))[:, None], EPS
    ).astype(np.float32)
    wnorm = np.sqrt(np.einsum("cd,cd->c", weight, weight, dtype=np.float64))
    wn = weight / np.maximum(wnorm[:, None], EPS).astype(np.float32)

    # label cosine computed on host in f64 (exact vs fp32 reference)
    wl = wn[label]  # [B, D]
    label_cos = np.einsum("bd,bd->b", xn.astype(np.float64), wl.astype(np.float64))
    lc2 = (SHIFT - S * label_cos).astype(np.float64)  # [B]
    lc2_pj = np.ascontiguousarray(lc2.reshape(BBLK, PB).T)  # [128, BBLK]

    xnt = np.ascontiguousarray((G * xn).T).astype(NP_FP8)  # [D, B]
    wnt = np.ascontiguousarray((G * wn).T.astype(NP_FP8))  # [D, C]

    in_maps = []
    for i in range(ncores):
        shard = np.ascontiguousarray(wnt[:, i * CS : (i + 1) * CS])
        in_maps.append({"wnt": shard, "xnt": xnt})
    return in_maps, lc2_pj


_NC_CACHE = {}


def _get_nc():
    if "nc" not in _NC_CACHE:
        _NC_CACHE["nc"] = build_nc()
    return _NC_CACHE["nc"]


def _install_ntff_hook():
    """The agent image's antenv lacks axon_hooks; shim it so trace=True can
    capture NTFF profiles via the ctypes hook in trn_agent_boot."""
    import sys
    import types

    try:
        from antenv.axon_hooks import get_axon_ntff_profile_hook  # noqa: F401
        return
    except ImportError:
        pass
    mod = types.ModuleType("antenv.axon_hooks")
    _state = {"hook": None}
    mod.set_axon_ntff_profile_hook = lambda h: _state.__setitem__("hook", h)
    mod.get_axon_ntff_profile_hook = lambda: _state["hook"]
    sys.modules["antenv.axon_hooks"] = mod
    import antenv

    antenv.axon_hooks = mod
    from trn_agent_boot.trn_boot import _ntff_profile_via_ctypes

    mod.set_axon_ntff_profile_hook(
        _ntff_profile_via_ctypes("/opt/axon/libaxon_pjrt.so")
    )
    # keep trace artifacts local (no external upload from this sandbox)
    import concourse.bass_utils as bu

    bu.upload_artifacts = lambda tmpdir: tmpdir


def finish_loss(results, lc2_pj):
    """Host epilogue: sum the 8 cores' per-group partials, log, add label
    term, mean."""
    Z = np.zeros((PB, BBLK), dtype=np.float64)
    for r in results:
        zp = r["zp"].astype(np.float64).reshape(PB, BBLK, NGROUPS)
        Z += zp.sum(axis=2)
    loss = float((np.log(Z) + lc2_pj).mean())
    return np.float32(loss)


def run(x, weight, label, trace=False):
    """Returns (loss_scalar, BassKernelResults)."""
    if trace:
        _install_ntff_hook()
    nc = _get_nc()
    in_maps, lc2_pj = prepare_inputs(x, weight, label)
    res = run_bass_kernel_spmd(
        nc, in_maps, core_ids=list(range(NCORES)), trace=trace
    )
    loss = finish_loss(res.results, lc2_pj)
    return loss, res


def kernel(x, weight, label, batch=None, **_ignored):
    loss, _ = run(x, weight, label, trace=False)
    return np.asarray(loss, dtype=np.float32)


# revision 6
# speedup vs baseline: 1.0542x; 1.0542x over previous
"""ArcFace loss kernel for 8 TRN2 NeuronCores (vocab/tensor-parallel).

reference:
    xn = normalize(x)               # [B, D]
    wn = normalize(weight)          # [C, D]
    logits = 64 * xn @ wn.T         # [B, C]
    loss = mean(CE(logits, label))

Strategy: shard classes C=100000 over 8 cores (12500 each, exact - no
padding). Host prepares normalized, transposed fp8(e4m3) operands scaled
by G=8 (so device cosines are 64*cos and the exp scale is 1); each core
computes its logit shard with TensorE fp8 DoubleRow matmuls (K=256 per
op) into fp32 PSUM and a fused exp+row-sum on ScalarE with a fixed shift
(logsumexp(l) = SHIFT + log(sum(exp(l - SHIFT))), exact since l <= 64).

v2 pipeline notes (from trace analysis of the 77.8us baseline):
  - ScalarE EXP stream is the pacer (~55us busy incl per-op overhead);
    critical path = preamble(5.7us fixed) + time-to-first-EXP + packed
    EXP stream + drain.
  - The 212-class tail chunk is processed as the FIRST group: it needs
    only 106KB of weights, so its 4 small EXPs start ~level with the
    first weight DMAs landing and fill the window while the first full
    1MB group streams in.
  - Warmup matmuls use a K=128 bf16 stationary (not K=1): the TRN2 PE
    p-state governor needs real array utilization to ramp 0.65->2.4GHz,
    and any PE idle gap during the ramp resets it (427ns/MM at MID).
  - DMA descriptor generation costs ~640ns per dma_start on the issuing
    engine; issue is spread over sync/vector/scalar/gpsimd so the first
    group's pieces are all in flight ~2 waves after the preamble.
  - Each core returns raw per-(row, bblock, group) partials [128, 28];
    the host sums group columns, so no on-device reduce/extra sync.
"""

import math
import numpy as np

import concourse.mybir as mybir
import concourse.tile as tile
from concourse import bacc
from concourse.bass_utils import run_bass_kernel_spmd

# Problem constants (hardcoded per harness contract).
B = 512
D = 512
C = 100000
S = 64.0
SHIFT = 20.0  # logsumexp shift; keeps Z ~1e-2 (HW Ln saturates below ~1e-19)
EPS = 1e-12
G = 8.0      # fp8 pre-scale on both operands: device cos' = G^2 * cos
NCORES = 8
CS = C // NCORES        # classes per core = 12500 (exact, no padding)
CHUNK = 512             # matmul moving free dim = one full PSUM bank
GROUP = 4               # psum banks per exp/accumulate group
PB = 128                # partitions
KSUB = D // PB          # 4 contraction subtiles of 128
BBLK = B // PB          # 4 batch blocks
TAIL = CS - 24 * CHUNK  # 212: ragged tail chunk, processed first
N_WARM = 9              # fat PE warm-up matmuls (K=128) to ramp the clock

F32 = mybir.dt.float32
BF16 = mybir.dt.bfloat16
FP8 = mybir.dt.float8e4
NP_FP8 = mybir.dt.np(FP8)
EXP_SCALE = S / (G * G)  # = 1.0

# groups: [tail 212] + six full groups of 4x512. (col0, ncols) per group.
GROUPS = [(24 * CHUNK, TAIL)] + [
    (g * GROUP * CHUNK, GROUP * CHUNK) for g in range(6)
]
NGROUPS = len(GROUPS)  # 7


def build_nc(ncores: int = NCORES):
    """Build the SPMD Bass graph."""
    nc = bacc.Bacc(
        "TRN2",
        target_bir_lowering=False,
        debug=False,
        num_devices=ncores,
    )

    wnt_ext = nc.dram_tensor("wnt", [D, CS], FP8, kind="ExternalInput")
    xnt_ext = nc.dram_tensor("xnt", [D, B], FP8, kind="ExternalInput")
    zp_ext = nc.dram_tensor("zp", [PB, BBLK * NGROUPS], F32, kind="ExternalOutput")

    with tile.TileContext(nc) as tc:
        with (
            tc.tile_pool(name="const", bufs=1) as cpool,
            tc.tile_pool(name="wpool", bufs=7) as wpool,
            tc.tile_pool(name="dpool", bufs=2) as dpool,
        ):
            # ---- SBUF tiles -------------------------------------------
            # exp bias (-SHIFT) as a per-partition vector
            negs = cpool.tile([PB, 1], F32)
            # x^T (normalized, G-scaled) as [128, KSUB, B]: d = ksub*128 + p
            xsb = cpool.tile([PB, KSUB, B], FP8)
            # warmup operands: real K=128 stationary so the PE ramps
            warm_s = cpool.tile([PB, PB], BF16)
            warm_m = cpool.tile([PB, CHUNK], BF16)
            # per (b-block, group) partial row-sums of exp(logit - SHIFT)
            partials = cpool.tile([PB, BBLK * NGROUPS], F32)

            # memsets go on gpsimd (its DMA issues come later); they must
            # not delay the first descriptor-gens on sync/vector/scalar.
            nc.gpsimd.memset(warm_s, 0.0)
            nc.gpsimd.memset(warm_m, 0.0)
            nc.gpsimd.memset(negs, -SHIFT)

            # ---- DMA issue plan ---------------------------------------
            # Descriptor generation costs ~0.62us per dma_start on the
            # issuing engine (only SP/sync, Activation/scalar, Pool/gpsimd
            # can issue) and each piece flows on one ring at ~21GB/s after
            # ~1.5us latency.  Explicit per-engine issue lists:
            #   scalar: 8 startup pieces only (tail weights + last quarter
            #           of group 1), then the ACT table load -> first EXP
            #           ~12.5us with zero further ScalarE DMA work.
            #   sync/gpsimd: xsb bb0 pieces first, then group 1 in 64KB
            #           pieces, remaining xsb, then groups 2..6 in 128KB
            #           pieces -- ordered to meet each group's EXP-stream
            #           deadline.
            wt_tiles = []
            for col0, ncols in GROUPS:
                wt_tiles.append(
                    wpool.tile(
                        [PB, KSUB, ncols], FP8, name="wt", tag="w",
                        padded_shape=[PB, KSUB, GROUP * CHUNK],
                    )
                )

            def issue_wt(gi, ks, h0, hn, eng):
                col0 = GROUPS[gi][0]
                eng.dma_start(
                    out=wt_tiles[gi][:, ks, h0 : h0 + hn],
                    in_=wnt_ext[
                        ks * PB : (ks + 1) * PB,
                        col0 + h0 : col0 + h0 + hn,
                    ],
                )

            def issue_xsb(ks, b0, bn, eng):
                eng.dma_start(
                    out=xsb[:, ks, b0 : b0 + bn],
                    in_=xnt_ext.rearrange("(ks p) b -> p ks b", p=PB)[
                        :, ks, b0 : b0 + bn
                    ],
                )

            # scalar: tail weights (4 x 27KB) + group1 c1536:2048 (4 x 64KB)
            for ks in range(KSUB):
                issue_wt(0, ks, 0, TAIL, nc.scalar)
            for ks in range(KSUB):
                issue_wt(1, ks, 1536, 512, nc.scalar)

            # sync handles ks0/ks1, gpsimd handles ks2/ks3, interleaved so
            # both engines' queues advance in the same priority order.
            sync_q = []
            gps_q = []
            for ks in (0, 1):
                sync_q.append(("xsb", ks, 0, PB))
            for ks in (2, 3):
                gps_q.append(("xsb", ks, 0, PB))
            for h0 in (0, 512, 1024):
                for ks in (0, 1):
                    sync_q.append(("wt", 1, ks, h0, 512))
                for ks in (2, 3):
                    gps_q.append(("wt", 1, ks, h0, 512))
            for ks in (0, 1):
                sync_q.append(("xsb", ks, PB, B - PB))
            for ks in (2, 3):
                gps_q.append(("xsb", ks, PB, B - PB))
            for gi in range(2, NGROUPS):
                for h0 in (0, 1024):
                    for ks in (0, 1):
                        sync_q.append(("wt", gi, ks, h0, 1024))
                    for ks in (2, 3):
                        gps_q.append(("wt", gi, ks, h0, 1024))

            for q, eng in ((sync_q, nc.sync), (gps_q, nc.gpsimd)):
                for item in q:
                    if item[0] == "wt":
                        issue_wt(item[1], item[2], item[3], item[4], eng)
                    else:
                        issue_xsb(item[1], item[2], item[3], eng)

            # ---- compute ----------------------------------------------
            with tc.tile_pool(name="psmain", bufs=2, space="PSUM") as pspool:
                # PE warm-up: full-array (K=128) matmuls so the p-state
                # governor ramps to 2.4GHz before the first real matmul.
                warm_ps = pspool.tile(
                    [PB, GROUP, CHUNK], F32, name="warm_ps", tag="ps",
                )
                for _ in range(N_WARM):
                    nc.tensor.matmul(
                        out=warm_ps[:, 0, :], lhsT=warm_s, rhs=warm_m,
                        start=True, stop=True,
                    )

                for gi, (col0, ncols) in enumerate(GROUPS):
                    nsub = math.ceil(ncols / CHUNK)
                    wt = wt_tiles[gi]
                    for bb in range(BBLK):
                        ps = pspool.tile(
                            [PB, nsub, CHUNK], F32, name="ps", tag="ps",
                            padded_shape=[PB, GROUP, CHUNK],
                        )
                        for k2 in range(KSUB // 2):
                            for sub in range(nsub):
                                cn = min(CHUNK, ncols - sub * CHUNK)
                                nc.tensor.matmul(
                                    out=ps[:, sub, :cn],
                                    lhsT=xsb[
                                        :, 2 * k2 : 2 * k2 + 2,
                                        bb * PB : (bb + 1) * PB,
                                    ],
                                    rhs=wt[
                                        :, 2 * k2 : 2 * k2 + 2,
                                        sub * CHUNK : sub * CHUNK + cn,
                                    ],
                                    start=(k2 == 0),
                                    stop=(k2 == KSUB // 2 - 1),
                                    perf_mode=mybir.MatmulPerfMode.DoubleRow,
                                )
                        dump = dpool.tile(
                            [PB, nsub, CHUNK], BF16, name="dump", tag="dump",
                            padded_shape=[PB, GROUP, CHUNK],
                        )
                        # exp(EXP_SCALE * cos' - SHIFT), accumulated per row.
                        # ragged tail group (nsub=1, 212 cols) reads exactly
                        # its columns; full groups read nsub*512.
                        if ncols % CHUNK == 0:
                            in_ap = ps[:, :, :]
                            out_ap = dump[:, :, :]
                        else:
                            in_ap = ps[:, 0, :ncols]
                            out_ap = dump[:, 0, :ncols]
                        nc.scalar.activation(
                            out=out_ap,
                            in_=in_ap,
                            func=mybir.ActivationFunctionType.Exp,
                            bias=negs,
                            scale=EXP_SCALE,
                            accum_out=partials[
                                :, bb * NGROUPS + gi : bb * NGROUPS + gi + 1
                            ],
                        )

            # raw partials out (host sums the group columns per row).
            # Split: groups 0..5's columns go out on sync while group 6's
            # EXPs still run; the final 4 columns ride a tiny DMA issued by
            # scalar right after its last accumulator read.
            pview = partials.rearrange("p (b g) -> p b g", b=BBLK)
            zview = zp_ext.rearrange("p (b g) -> p b g", b=BBLK)
            nc.sync.dma_start(
                out=zview[:, :, 0 : NGROUPS - 1], in_=pview[:, :, 0 : NGROUPS - 1]
            )
            nc.scalar.dma_start(
                out=zview[:, :, NGROUPS - 1 : NGROUPS],
                in_=pview[:, :, NGROUPS - 1 : NGROUPS],
            )

    nc.finalize()
    return nc


def prepare_inputs(x, weight, label, ncores: int = NCORES):
    """Host-side prep: normalize, transpose, G-scale, cast fp8, shard.

    Returns (in_maps, lc2) where lc2[p, j] = SHIFT - S*cos(x_b, w_label_b)
    for b = j*128 + p."""
    x = np.asarray(x, dtype=np.float32)
    weight = np.asarray(weight, dtype=np.float32)
    label = np.asarray(label).astype(np.int64)

    xn = x / np.maximum(
        np.sqrt(np.einsum("bd,bd->b", x, x, dtype=np.float64))[:, None], EPS
    ).astype(np.float32)
    wnorm = np.sqrt(np.einsum("cd,cd->c", weight, weight, dtype=np.float64))
    wn = weight / np.maximum(wnorm[:, None], EPS).astype(np.float32)

    # label cosine computed on host in f64 (exact vs fp32 reference)
    wl = wn[label]  # [B, D]
    label_cos = np.einsum("bd,bd->b", xn.astype(np.float64), wl.astype(np.float64))
    lc2 = (SHIFT - S * label_cos).astype(np.float64)  # [B]
    lc2_pj = np.ascontiguousarray(lc2.reshape(BBLK, PB).T)  # [128, BBLK]

    xnt = np.ascontiguousarray((G * xn).T).astype(NP_FP8)  # [D, B]
    wnt = np.ascontiguousarray((G * wn).T.astype(NP_FP8))  # [D, C]

    in_maps = []
    for i in range(ncores):
        shard = np.ascontiguousarray(wnt[:, i * CS : (i + 1) * CS])
        in_maps.append({"wnt": shard, "xnt": xnt})
    return in_maps, lc2_pj


_NC_CACHE = {}


def _get_nc():
    if "nc" not in _NC_CACHE:
        _NC_CACHE["nc"] = build_nc()
    return _NC_CACHE["nc"]


def _install_ntff_hook():
    """The agent image's antenv lacks axon_hooks; shim it so trace=True can
    capture NTFF profiles via the ctypes hook in trn_agent_boot."""
    import sys
    import types

    try:
        from antenv.axon_hooks import get_axon_ntff_profile_hook  # noqa: F401
        return
    except ImportError:
        pass
    mod = types.ModuleType("antenv.axon_hooks")
    _state = {"hook": None}
    mod.set_axon_ntff_profile_hook = lambda h: _state.__setitem__("hook", h)
    mod.get_axon_ntff_profile_hook = lambda: _state["hook"]
    sys.modules["antenv.axon_hooks"] = mod
    import antenv

    antenv.axon_hooks = mod
    from trn_agent_boot.trn_boot import _ntff_profile_via_ctypes

    mod.set_axon_ntff_profile_hook(
        _ntff_profile_via_ctypes("/opt/axon/libaxon_pjrt.so")
    )
    # keep trace artifacts local (no external upload from this sandbox)
    import concourse.bass_utils as bu

    bu.upload_artifacts = lambda tmpdir: tmpdir


def finish_loss(results, lc2_pj):
    """Host epilogue: sum the 8 cores' per-group partials, log, add label
    term, mean."""
    Z = np.zeros((PB, BBLK), dtype=np.float64)
    for r in results:
        zp = r["zp"].astype(np.float64).reshape(PB, BBLK, NGROUPS)
        Z += zp.sum(axis=2)
    loss = float((np.log(Z) + lc2_pj).mean())
    return np.float32(loss)


def run(x, weight, label, trace=False):
    """Returns (loss_scalar, BassKernelResults)."""
    if trace:
        _install_ntff_hook()
    nc = _get_nc()
    in_maps, lc2_pj = prepare_inputs(x, weight, label)
    res = run_bass_kernel_spmd(
        nc, in_maps, core_ids=list(range(NCORES)), trace=trace
    )
    loss = finish_loss(res.results, lc2_pj)
    return loss, res


def kernel(x, weight, label, batch=None, **_ignored):
    loss, _ = run(x, weight, label, trace=False)
    return np.asarray(loss, dtype=np.float32)


# revision 10
# speedup vs baseline: 1.1019x; 1.0452x over previous
"""ArcFace loss kernel for 8 TRN2 NeuronCores (vocab/tensor-parallel).

reference:
    xn = normalize(x)               # [B, D]
    wn = normalize(weight)          # [C, D]
    logits = 64 * xn @ wn.T         # [B, C]
    loss = mean(CE(logits, label))

Strategy: shard classes C=100000 over 8 cores (12500 each, exact - no
padding). Host prepares normalized, transposed fp8(e4m3) operands scaled
by G=8 (so device cosines are 64*cos and the exp scale is 1); each core
computes its logit shard with TensorE fp8 DoubleRow matmuls (K=256 per
op) into fp32 PSUM and a fused exp+row-sum on ScalarE with a fixed shift
(logsumexp(l) = SHIFT + log(sum(exp(l - SHIFT))), exact since l <= 64).

v2 pipeline notes (from trace analysis of the 77.8us baseline):
  - ScalarE EXP stream is the pacer (~55us busy incl per-op overhead);
    critical path = preamble(5.7us fixed) + time-to-first-EXP + packed
    EXP stream + drain.
  - The 212-class tail chunk is processed as the FIRST group: it needs
    only 106KB of weights, so its 4 small EXPs start ~level with the
    first weight DMAs landing and fill the window while the first full
    1MB group streams in.
  - Warmup matmuls use a K=128 bf16 stationary (not K=1): the TRN2 PE
    p-state governor needs real array utilization to ramp 0.65->2.4GHz,
    and any PE idle gap during the ramp resets it (427ns/MM at MID).
  - DMA descriptor generation costs ~640ns per dma_start on the issuing
    engine; issue is spread over sync/vector/scalar/gpsimd so the first
    group's pieces are all in flight ~2 waves after the preamble.
  - Each core returns raw per-(row, bblock, group) partials [128, 28];
    the host sums group columns, so no on-device reduce/extra sync.
"""

import math
import numpy as np

import concourse.mybir as mybir
import concourse.tile as tile
from concourse import bacc
from concourse.bass_utils import run_bass_kernel_spmd

# Problem constants (hardcoded per harness contract).
B = 512
D = 512
C = 100000
S = 64.0
SHIFT = 20.0  # logsumexp shift; keeps Z ~1e-2 (HW Ln saturates below ~1e-19)
EPS = 1e-12
G = 8.0      # fp8 pre-scale on both operands: device cos' = G^2 * cos
NCORES = 8
CS = C // NCORES        # classes per core = 12500 (exact, no padding)
CHUNK = 512             # matmul moving free dim = one full PSUM bank
GROUP = 4               # psum banks per exp/accumulate group
PB = 128                # partitions
KSUB = D // PB          # 4 contraction subtiles of 128
BBLK = B // PB          # 4 batch blocks
TAIL = CS - 24 * CHUNK  # 212: ragged tail chunk, processed first
N_WARM = 13             # fat PE warm-up matmuls (K=128) to ramp the clock

F32 = mybir.dt.float32
BF16 = mybir.dt.bfloat16
FP8 = mybir.dt.float8e4
NP_FP8 = mybir.dt.np(FP8)
EXP_SCALE = S / (G * G)  # = 1.0

# groups: [tail 212] + six full groups of 4x512. (col0, ncols) per group.
GROUPS = [(24 * CHUNK, TAIL)] + [
    (g * GROUP * CHUNK, GROUP * CHUNK) for g in range(6)
]
NGROUPS = len(GROUPS)  # 7


def build_nc(ncores: int = NCORES):
    """Build the SPMD Bass graph."""
    nc = bacc.Bacc(
        "TRN2",
        target_bir_lowering=False,
        debug=False,
        num_devices=ncores,
    )

    wnt_ext = nc.dram_tensor("wnt", [D, CS], FP8, kind="ExternalInput")
    xnt_ext = nc.dram_tensor("xnt", [D, B], FP8, kind="ExternalInput")
    zp_ext = nc.dram_tensor("zp", [PB, BBLK * NGROUPS], F32, kind="ExternalOutput")

    with tile.TileContext(nc) as tc:
        with (
            tc.tile_pool(name="const", bufs=1) as cpool,
            tc.tile_pool(name="wpool", bufs=7) as wpool,
            tc.tile_pool(name="dpool", bufs=2) as dpool,
        ):
            # ---- SBUF tiles -------------------------------------------
            # exp bias (-SHIFT) as a per-partition vector
            negs = cpool.tile([PB, 1], F32)
            # x^T (normalized, G-scaled) as [128, KSUB, B]: d = ksub*128 + p
            xsb = cpool.tile([PB, KSUB, B], FP8)
            # warmup operands: real K=128 stationary so the PE ramps
            warm_s = cpool.tile([PB, PB], BF16)
            warm_m = cpool.tile([PB, CHUNK], BF16)
            # per (b-block, group) partial row-sums of exp(logit - SHIFT)
            partials = cpool.tile([PB, BBLK * NGROUPS], F32)

            # memsets go on vector (it cannot issue DMAs and is otherwise
            # idle); the three DMA-capable engines start descriptor-gen
            # with their very first instruction.
            nc.vector.memset(warm_s, 0.0)
            nc.vector.memset(warm_m, 0.0)
            nc.vector.memset(negs, -SHIFT)

            # ---- DMA issue plan ---------------------------------------
            # Descriptor generation costs ~0.62us per dma_start on the
            # issuing engine (only SP/sync, Activation/scalar, Pool/gpsimd
            # can issue) and each piece flows on one ring at ~21GB/s after
            # ~1.5us latency.  Explicit per-engine issue lists:
            #   scalar: 8 startup pieces only (tail weights + last quarter
            #           of group 1), then the ACT table load -> first EXP
            #           ~12.5us with zero further ScalarE DMA work.
            #   sync/gpsimd: xsb bb0 pieces first, then group 1 in 64KB
            #           pieces, remaining xsb, then groups 2..6 in 128KB
            #           pieces -- ordered to meet each group's EXP-stream
            #           deadline.
            wt_tiles = []
            for col0, ncols in GROUPS:
                wt_tiles.append(
                    wpool.tile(
                        [PB, KSUB, ncols], FP8, name="wt", tag="w",
                        padded_shape=[PB, KSUB, GROUP * CHUNK],
                    )
                )

            def issue_wt(gi, ks, h0, hn, eng):
                col0 = GROUPS[gi][0]
                eng.dma_start(
                    out=wt_tiles[gi][:, ks, h0 : h0 + hn],
                    in_=wnt_ext[
                        ks * PB : (ks + 1) * PB,
                        col0 + h0 : col0 + h0 + hn,
                    ],
                )

            def issue_xsb(ks, b0, bn, eng):
                eng.dma_start(
                    out=xsb[:, ks, b0 : b0 + bn],
                    in_=xnt_ext.rearrange("(ks p) b -> p ks b", p=PB)[
                        :, ks, b0 : b0 + bn
                    ],
                )

            # scalar: tail weights (4 x 27KB) + group1 c1536:2048 (4 x 64KB)
            for ks in range(KSUB):
                issue_wt(0, ks, 0, TAIL, nc.scalar)
            for ks in range(KSUB):
                issue_wt(1, ks, 1536, 512, nc.scalar)

            # sync handles ks0/ks1, gpsimd handles ks2/ks3, interleaved so
            # both engines' queues advance in the same priority order:
            # xsb lower batch half (unblocks tail bb0/bb1), group 1 weights,
            # xsb upper half (tail bb2/bb3 run while g1 still streams),
            # then groups 2..6.
            sync_q = []
            gps_q = []
            for ks in (0, 1):
                sync_q.append(("xsb", ks, 0, 2 * PB))
            for ks in (2, 3):
                gps_q.append(("xsb", ks, 0, 2 * PB))
            for h0 in (0, 512, 1024):
                for ks in (0, 1):
                    sync_q.append(("wt", 1, ks, h0, 512))
                for ks in (2, 3):
                    gps_q.append(("wt", 1, ks, h0, 512))
            for ks in (0, 1):
                sync_q.append(("xsb", ks, 2 * PB, B - 2 * PB))
            for ks in (2, 3):
                gps_q.append(("xsb", ks, 2 * PB, B - 2 * PB))
            for gi in range(2, NGROUPS):
                for h0 in (0, 1024):
                    for ks in (0, 1):
                        sync_q.append(("wt", gi, ks, h0, 1024))
                    for ks in (2, 3):
                        gps_q.append(("wt", gi, ks, h0, 1024))

            for q, eng in ((sync_q, nc.sync), (gps_q, nc.gpsimd)):
                for item in q:
                    if item[0] == "wt":
                        issue_wt(item[1], item[2], item[3], item[4], eng)
                    else:
                        issue_xsb(item[1], item[2], item[3], eng)

            # ---- compute ----------------------------------------------
            with tc.tile_pool(name="psmain", bufs=2, space="PSUM") as pspool:
                # PE warm-up: full-array (K=128) matmuls so the p-state
                # governor ramps to 2.4GHz before the first real matmul.
                warm_ps = pspool.tile(
                    [PB, GROUP, CHUNK], F32, name="warm_ps", tag="ps",
                )
                for _ in range(N_WARM):
                    nc.tensor.matmul(
                        out=warm_ps[:, 0, :], lhsT=warm_s, rhs=warm_m,
                        start=True, stop=True,
                    )

                for gi, (col0, ncols) in enumerate(GROUPS):
                    nsub = math.ceil(ncols / CHUNK)
                    wt = wt_tiles[gi]
                    for bb in range(BBLK):
                        ps = pspool.tile(
                            [PB, nsub, CHUNK], F32, name="ps", tag="ps",
                            padded_shape=[PB, GROUP, CHUNK],
                        )
                        for k2 in range(KSUB // 2):
                            for sub in range(nsub):
                                cn = min(CHUNK, ncols - sub * CHUNK)
                                nc.tensor.matmul(
                                    out=ps[:, sub, :cn],
                                    lhsT=xsb[
                                        :, 2 * k2 : 2 * k2 + 2,
                                        bb * PB : (bb + 1) * PB,
                                    ],
                                    rhs=wt[
                                        :, 2 * k2 : 2 * k2 + 2,
                                        sub * CHUNK : sub * CHUNK + cn,
                                    ],
                                    start=(k2 == 0),
                                    stop=(k2 == KSUB // 2 - 1),
                                    perf_mode=mybir.MatmulPerfMode.DoubleRow,
                                )
                        dump = dpool.tile(
                            [PB, nsub, CHUNK], BF16, name="dump", tag="dump",
                            padded_shape=[PB, GROUP, CHUNK],
                        )
                        # exp(EXP_SCALE * cos' - SHIFT), accumulated per row.
                        # ragged tail group (nsub=1, 212 cols) reads exactly
                        # its columns; full groups read nsub*512.
                        if ncols % CHUNK == 0:
                            in_ap = ps[:, :, :]
                            out_ap = dump[:, :, :]
                        else:
                            in_ap = ps[:, 0, :ncols]
                            out_ap = dump[:, 0, :ncols]
                        nc.scalar.activation(
                            out=out_ap,
                            in_=in_ap,
                            func=mybir.ActivationFunctionType.Exp,
                            bias=negs,
                            scale=EXP_SCALE,
                            accum_out=partials[
                                :, bb * NGROUPS + gi : bb * NGROUPS + gi + 1
                            ],
                        )

            # raw partials out (host sums the group columns per row),
            # issued by scalar itself: descriptor-gen starts right after
            # its last accumulator read with no cross-engine semaphore.
            nc.scalar.dma_start(out=zp_ext[:], in_=partials)

    nc.finalize()
    return nc


def prepare_inputs(x, weight, label, ncores: int = NCORES):
    """Host-side prep: normalize, transpose, G-scale, cast fp8, shard.

    Returns (in_maps, lc2) where lc2[p, j] = SHIFT - S*cos(x_b, w_label_b)
    for b = j*128 + p."""
    x = np.asarray(x, dtype=np.float32)
    weight = np.asarray(weight, dtype=np.float32)
    label = np.asarray(label).astype(np.int64)

    xn = x / np.maximum(
        np.sqrt(np.einsum("bd,bd->b", x, x, dtype=np.float64))[:, None], EPS
    ).astype(np.float32)
    wnorm = np.sqrt(np.einsum("cd,cd->c", weight, weight, dtype=np.float64))
    wn = weight / np.maximum(wnorm[:, None], EPS).astype(np.float32)

    # label cosine computed on host in f64 (exact vs fp32 reference)
    wl = wn[label]  # [B, D]
    label_cos = np.einsum("bd,bd->b", xn.astype(np.float64), wl.astype(np.float64))
    lc2 = (SHIFT - S * label_cos).astype(np.float64)  # [B]
    lc2_pj = np.ascontiguousarray(lc2.reshape(BBLK, PB).T)  # [128, BBLK]

    xnt = np.ascontiguousarray((G * xn).T).astype(NP_FP8)  # [D, B]
    wnt = np.ascontiguousarray((G * wn).T.astype(NP_FP8))  # [D, C]

    in_maps = []
    for i in range(ncores):
        shard = np.ascontiguousarray(wnt[:, i * CS : (i + 1) * CS])
        in_maps.append({"wnt": shard, "xnt": xnt})
    return in_maps, lc2_pj


_NC_CACHE = {}


def _get_nc():
    if "nc" not in _NC_CACHE:
        _NC_CACHE["nc"] = build_nc()
    return _NC_CACHE["nc"]


def _install_ntff_hook():
    """The agent image's antenv lacks axon_hooks; shim it so trace=True can
    capture NTFF profiles via the ctypes hook in trn_agent_boot."""
    import sys
    import types

    try:
        from antenv.axon_hooks import get_axon_ntff_profile_hook  # noqa: F401
        return
    except ImportError:
        pass
    mod = types.ModuleType("antenv.axon_hooks")
    _state = {"hook": None}
    mod.set_axon_ntff_profile_hook = lambda h: _state.__setitem__("hook", h)
    mod.get_axon_ntff_profile_hook = lambda: _state["hook"]
    sys.modules["antenv.axon_hooks"] = mod
    import antenv

    antenv.axon_hooks = mod
    from trn_agent_boot.trn_boot import _ntff_profile_via_ctypes

    mod.set_axon_ntff_profile_hook(
        _ntff_profile_via_ctypes("/opt/axon/libaxon_pjrt.so")
    )
    # keep trace artifacts local (no external upload from this sandbox)
    import concourse.bass_utils as bu

    bu.upload_artifacts = lambda tmpdir: tmpdir


def finish_loss(results, lc2_pj):
    """Host epilogue: sum the 8 cores' per-group partials, log, add label
    term, mean."""
    Z = np.zeros((PB, BBLK), dtype=np.float64)
    for r in results:
        zp = r["zp"].astype(np.float64).reshape(PB, BBLK, NGROUPS)
        Z += zp.sum(axis=2)
    loss = float((np.log(Z) + lc2_pj).mean())
    return np.float32(loss)


def run(x, weight, label, trace=False):
    """Returns (loss_scalar, BassKernelResults)."""
    if trace:
        _install_ntff_hook()
    nc = _get_nc()
    in_maps, lc2_pj = prepare_inputs(x, weight, label)
    res = run_bass_kernel_spmd(
        nc, in_maps, core_ids=list(range(NCORES)), trace=trace
    )
    loss = finish_loss(res.results, lc2_pj)
    return loss, res


def kernel(x, weight, label, batch=None, **_ignored):
    loss, _ = run(x, weight, label, trace=False)
    return np.asarray(loss, dtype=np.float32)
